# revision 41
# baseline (speedup 1.0000x reference)
"""Contrastive-loss (InfoNCE / softmax-CE) kernel for 8 Trainium2 NeuronCores.

reference semantics:
    scores = feature @ anchor.T          # [B, CLS]
    loss   = mean_b( logsumexp(scores[b]) - scores[b, target[b]] )

Strategy (data-parallel, per sharding hint):
  - shard feature/_target along batch across 8 cores (2048 rows each),
    replicate anchor.
  - host packs transposed fp8-e4m3 layouts (featX [NGRP, KT, P, GCOLS] per
    core, anchorT [FEAT, CLS], tgtX [P, MT]) so the contraction dim lands on
    SBUF partitions, every device DMA reads a contiguous HBM block, and
    input bytes are quartered vs fp32 (6.3 MB/core). With fp32 PSUM
    accumulation the fp8 quantization error on the mean NLL is ~7e-4
    relative — 28x inside the 2e-2 gate (fp16 fallback: ~7e-7). Only ~21
    DMA instructions total (HW pays ~1us fixed per DMA).
  - matmuls run fp8 at 1 cyc/row. Both DoubleRow variants were implemented
    and HW-verified correct but measure slower (see CL_DR comment): the
    per-matmul LDWEIGHTS reload dominates once the matmul itself is halved.
  - device: 32 PE-clock warm-up matmuls run while the first operands stream
    in (the HAM clock gate needs ~3.4us of sustained activity to release);
    then per sub-group of 2 batch m-tiles, kt-outer accumulation into 2
    double-buffered PSUM tiles (2 banks each) so the PE chases the incoming
    DMA stream with no bubbles; drain = row-max (DVE) -> exp+sum (ACT, fused
    accumulate) -> target-score extraction via iota==target mask (DVE, fused
    accumulate). One batched Ln + per-m NLL combine at the tail.
  - host: mean over all 16384 rows (the scalar all-reduce).

Measured (loop-differential, 8 cores): ~122 us/iteration, within ~13% of
the 107 us fp16 matmul roofline (2048x1000x2048 MACs/core at 2.4 GHz).

Matmul dtype knob (CL_MM_DTYPE): f16 (default), bf16, f32r, f32.
"""

import os
import sys
from contextlib import ExitStack

import numpy as np

for _p in ("/opt/trn_rl_repo",):
    if os.path.isdir(_p) and _p not in sys.path:
        sys.path.insert(0, _p)

import concourse.bass as bass
import concourse.bacc as bacc
import concourse.mybir as mybir
import concourse.tile as tile

B, CLS, FEAT = 16384, 1000, 2048
NCORES = 8
BPC = B // NCORES          # 2048 batch rows per core
P = 128                    # partitions
KT = FEAT // P             # 16 contraction tiles
MT = BPC // P              # 16 batch m-tiles per core
GCOLS = 512                # feature columns per streamed DMA group
MPG = GCOLS // P           # 4 m-tiles per DMA group
NGRP = MT // MPG           # 4 DMA groups
SUB = 2                    # m-tiles per PSUM sub-group (2 tiles x 2 banks)
N0 = 512                   # class split: one PSUM bank of fp32
N1 = CLS - N0              # 488

MM_DTYPE = os.environ.get("CL_MM_DTYPE", "f8")
WARMUP_MM = int(os.environ.get("CL_WARMUP_MM", "32"))  # PE clock pre-warm
FIRST_QUAD = os.environ.get("CL_FIRST_QUAD", "0") == "1"
# fp8 DoubleRow (2 fp8 MACs per PE cell per cycle). CL_DR: 0=off (default),
# 1=plain DoubleRow, 2=DoubleRowSwInterleave (host pre-interleaved weights).
# Both verified correct on HW but NEITHER wins here: mode 1 measures 584us
# per iter (plane-separated weights AP hits a slow LDWEIGHTS lowering) and
# mode 2 measures 120us vs 112.5us for plain fp8 — with the matmul halved to
# ~107ns, the per-matmul 256-column LDWEIGHTS reload no longer hides behind
# the previous matmul and becomes the PE rate limiter ("LDWEIGHTS pays +72%").
DR_MODE = int(os.environ.get("CL_DR", "0"))


def _mm_dt(mm_dtype: str):
    return {
        "f8": mybir.dt.float8e4,
        "f16": mybir.dt.float16,
        "bf16": mybir.dt.bfloat16,
        "f32": mybir.dt.float32,
        "f32r": mybir.dt.float32r,
    }[mm_dtype]


def _np_mm(mm_dtype: str):
    if mm_dtype == "f8":
        import ml_dtypes

        return np.dtype(ml_dtypes.float8_e4m3)
    if mm_dtype == "f16":
        return np.dtype(np.float16)
    if mm_dtype == "bf16":
        import ml_dtypes

        return np.dtype(ml_dtypes.bfloat16)
    return np.dtype(np.float32)


def build_program(mm_dtype: str = MM_DTYPE, reps: int = 1,
                  loop_iters: int = 1) -> bass.Bass:
    """Build the per-core Bass/Tile program (SPMD: same program on all cores).

    reps > 1 repeats the full body (including all DMAs) for differential
    device-time measurement."""
    f32 = mybir.dt.float32
    mdt = _mm_dt(mm_dtype)

    nc = bacc.Bacc(None, target_bir_lowering=False, debug=False)
    dr2 = DR_MODE == 2 and mdt in (mybir.dt.float8e4, mybir.dt.float8e5)
    # featX[g, kt, p, c] = feature_core[g*GCOLS + c, kt*P + p] (pre-packed on
    # host) so every per-(g,kt) DMA reads one fully contiguous HBM block.
    # In SwInterleave mode the kt axis is kt-pairs and c interleaves the pair
    # (A/B per batch column, columns reversed per 128-wide m-tile).
    fkt = KT // 2 if dr2 else KT
    fcols = 2 * GCOLS if dr2 else GCOLS
    featX = nc.dram_tensor("featX", [NGRP, fkt, P, fcols], mdt,
                           kind="ExternalInput")
    anchorT = nc.dram_tensor("anchorT", [FEAT, CLS], mdt, kind="ExternalInput")
    # tgtX[p, m] = target[m*P + p]; same layout for the nll output
    tgtX = nc.dram_tensor("tgtX", [P, MT], f32, kind="ExternalInput")
    nllX = nc.dram_tensor("nll", [P, MT], f32, kind="ExternalOutput")

    fview = featX.ap().rearrange("g kt p c -> p g kt c")       # [128, 4, fkt, fcols]
    aview = anchorT.ap().rearrange("(kt p) c -> p kt c", p=P)  # [128, 16, 1000]

    with tile.TileContext(nc) as tc, ExitStack() as ctx:
        singles = ctx.enter_context(tc.tile_pool(name="singles", bufs=1))
        # 1/2-byte dtypes: whole feature resident (4 slabs); 4-byte: stream
        # with one-group-ahead prefetch (2 slabs)
        feat_bufs = 2 if mdt in (mybir.dt.float32, mybir.dt.float32r) else 4
        feats = ctx.enter_context(tc.tile_pool(name="feats", bufs=feat_bufs))
        psum = ctx.enter_context(tc.tile_pool(name="psum", bufs=4, space="PSUM"))
        scratch = ctx.enter_context(tc.tile_pool(name="scratch", bufs=2))

        # iota row 0..CLS-1 (exact in f32), replicated on every partition
        iota_i = singles.tile([P, CLS], mybir.dt.int32)
        nc.gpsimd.iota(iota_i, pattern=[[1, CLS]], base=0, channel_multiplier=0)
        iota_f = singles.tile([P, CLS], f32)
        nc.vector.tensor_copy(out=iota_f, in_=iota_i)

        if loop_iters > 1:
            assert reps == 1
            with tc.For_i(0, loop_iters, 1):
                _loss_body(nc, tc, mdt, fview, aview, tgtX, nllX, iota_f,
                           singles, feats, psum, scratch)
        else:
            for _rep in range(reps):
                _loss_body(nc, tc, mdt, fview, aview, tgtX, nllX, iota_f,
                           singles, feats, psum, scratch)

    return nc


def _loss_body(nc, tc, mdt, fview, aview, tgt, nll, iota_f,
               singles, feats, psum, scratch):
    f32 = mybir.dt.float32
    fp8 = mdt in (mybir.dt.float8e4, mybir.dt.float8e5)
    dr1 = DR_MODE == 1 and fp8
    dr2 = DR_MODE == 2 and fp8

    # anchor resident in SBUF, loaded per-kt so matmuls can start early.
    # DoubleRow needs a %16==0 byte step between the kt-pair planes, so pad
    # the class dim to 1024 (the pad is never read).
    CA = 1024 if (dr1 or dr2) else CLS
    anchor_sb = singles.tile([P, KT, CA], mdt, name="anchor_sb")
    # per-row target index as f32; column m holds rows [m*128, (m+1)*128)
    tgt_sb = singles.tile([P, MT], f32, name="tgt_sb")
    # per-m-tile stats, finalized in one tail op pair
    nmx_all = singles.tile([P, MT], f32, name="nmx_all")    # -max per row
    sume_all = singles.tile([P, MT], f32, name="sume_all")  # sum exp(s-max)
    st_all = singles.tile([P, MT], f32, name="st_all")      # s_target
    nll_sb = singles.tile([P, MT], f32, name="nll_sb")

    # feature DMA groups: [128, kt, GCOLS] each; group 0 interleaved with the
    # anchor chunks so the first matmuls unblock as early as possible.
    gtiles = [None] * NGRP

    fkt = KT // 2 if dr2 else KT
    fcols = 2 * GCOLS if dr2 else GCOLS

    def issue_group(g):
        slab = feats.tile([P, fkt, fcols], mdt, name="slab", tag="slab")
        gtiles[g] = slab
        if g > 0:
            # later groups stream well ahead of use: one DMA per group
            nc.sync.dma_start(out=slab, in_=fview[:, g])
            return
        # group 0 + anchor interleaved: fine-grained at the start (the PE
        # chases this stream during the cold window), fused quads later
        if dr2:
            # first matmul needs anchor planes kt0+kt1 and the first half of
            # the interleaved kt2=0 slab row
            nc.sync.dma_start(out=anchor_sb[:, 0, 0:N0], in_=aview[:, 0, 0:N0])
            nc.sync.dma_start(out=anchor_sb[:, 1, 0:N0], in_=aview[:, 1, 0:N0])
            nc.sync.dma_start(out=slab[:, 0, 0:GCOLS], in_=fview[:, 0, 0, 0:GCOLS])
            nc.sync.dma_start(out=anchor_sb[:, 0, N0:CLS], in_=aview[:, 0, N0:CLS])
            nc.sync.dma_start(out=anchor_sb[:, 1, N0:CLS], in_=aview[:, 1, N0:CLS])
            nc.sync.dma_start(out=slab[:, 0, GCOLS:2 * GCOLS],
                              in_=fview[:, 0, 0, GCOLS:2 * GCOLS])
            nc.sync.dma_start(out=tgt_sb, in_=tgt.ap())
            for kt2 in (1, 2, 3):
                nc.sync.dma_start(out=anchor_sb[:, 2 * kt2 : 2 * kt2 + 2, 0:CLS],
                                  in_=aview[:, 2 * kt2 : 2 * kt2 + 2, :])
                nc.sync.dma_start(out=slab[:, kt2, :], in_=fview[:, 0, kt2, :])
            nc.sync.dma_start(out=anchor_sb[:, 8:12, 0:CLS], in_=aview[:, 8:12, :])
            nc.sync.dma_start(out=slab[:, 4:8, :], in_=fview[:, 0, 4:8, :])
            nc.sync.dma_start(out=anchor_sb[:, 12:16, 0:CLS],
                              in_=aview[:, 12:16, :])
            return
        nc.sync.dma_start(out=anchor_sb[:, 0, 0:N0], in_=aview[:, 0, 0:N0])
        nc.sync.dma_start(out=slab[:, 0, 0:2 * P], in_=fview[:, 0, 0, 0:2 * P])
        nc.sync.dma_start(out=anchor_sb[:, 0, N0:CLS], in_=aview[:, 0, N0:CLS])
        nc.sync.dma_start(out=slab[:, 0, 2 * P:GCOLS],
                          in_=fview[:, 0, 0, 2 * P:GCOLS])
        # tiny; needed by the first drain (~15us in)
        nc.sync.dma_start(out=tgt_sb, in_=tgt.ap())
        for kt in (1, 2, 3):
            nc.sync.dma_start(out=anchor_sb[:, kt, 0:CLS], in_=aview[:, kt, :])
            nc.sync.dma_start(out=slab[:, kt, :], in_=fview[:, 0, kt, :])
        for kt in (4, 8, 12):
            nc.sync.dma_start(out=anchor_sb[:, kt : kt + 4, 0:CLS],
                              in_=aview[:, kt : kt + 4, :])
            nc.sync.dma_start(out=slab[:, kt : kt + 4, :],
                              in_=fview[:, 0, kt : kt + 4, :])

    issue_group(0)
    issue_group(1)

    # PE clock pre-warm: the HAM clock gate releases only after ~3.4us of
    # sustained PE activity. Dummy back-to-back matmuls on a memset tile keep
    # the PE busy while the first real operands stream in, so the real matmul
    # stream runs at full clock from the start. Their PSUM writes land in the
    # first pool buffer and are discarded by the first real start=True matmul.
    if WARMUP_MM > 0:
        wtile = singles.tile([P, P], mdt, name="warm_src")
        nc.vector.memset(wtile, 0.0)
        warm_ps = psum.tile([P, 2, N0], f32, name="warm_ps", tag="ps")
        for _ in range(WARMUP_MM):
            nc.tensor.matmul(
                warm_ps[:, 0, 0:P], wtile, wtile, start=True, stop=True
            )

    # sub-group sizes: first quad rides the cold anchor/g0 DMA stream with
    # enough parallel work to keep the PE busy; pairs after that
    if FIRST_QUAD:
        sub_sizes = [4] + [SUB] * ((MT - 4) // SUB)
    else:
        sub_sizes = [SUB] * (MT // SUB)
    m0 = 0
    for s, sub in enumerate(sub_sizes):    # sub-groups of `sub` m-tiles
        g = m0 // MPG
        if m0 % MPG == 0 and g >= 1 and g + 1 < NGRP:
            issue_group(g + 1)          # prefetch one group ahead

        slab = gtiles[g]
        ps_list = [
            psum.tile([P, 2, N0], f32, name="ps", tag="ps") for _ in range(sub)
        ]
        # kt-outer over the sub-group: each arriving anchor/slab chunk
        # unlocks sub*2 matmuls, so the PE chases the DMA stream. With
        # DoubleRow modes, adjacent kt pairs form the packed operands: the
        # rhs as [Ki, 2, N] plane pairs; the weights as plane pairs (mode 1)
        # or as the host-pre-interleaved contiguous [Ki, 2*M] row (mode 2).
        kstep = 2 if (dr1 or dr2) else 1
        pm = (mybir.MatmulPerfMode.DoubleRow if dr1
              else mybir.MatmulPerfMode.DoubleRowSwInterleave if dr2
              else None)
        for kt in range(0, KT, kstep):
            for j in range(sub):
                m = m0 + j
                mb = (m * P) % GCOLS
                if dr2:
                    lhsT = slab[:, kt // 2, 2 * mb : 2 * mb + 2 * P]
                elif dr1:
                    lhsT = slab[:, kt : kt + 2, mb : mb + P]
                else:
                    lhsT = slab[:, kt, mb : mb + P]
                rhs0 = (anchor_sb[:, kt : kt + 2, 0:N0] if kstep == 2
                        else anchor_sb[:, kt, 0:N0])
                rhs1 = (anchor_sb[:, kt : kt + 2, N0:CLS] if kstep == 2
                        else anchor_sb[:, kt, N0:CLS])
                nc.tensor.matmul(
                    ps_list[j][:, 0, :],
                    lhsT,
                    rhs0,
                    start=(kt == 0),
                    stop=(kt + kstep >= KT),
                    perf_mode=pm,
                )
                nc.tensor.matmul(
                    ps_list[j][:, 1, 0:N1],
                    lhsT,
                    rhs1,
                    start=(kt == 0),
                    stop=(kt + kstep >= KT),
                    perf_mode=pm,
                )

        for j in range(sub):
            m = m0 + j
            flat = ps_list[j].rearrange("p a b -> p (a b)")[:, 0:CLS]  # [128,1000]

            # -max(scores) per row (DVE)
            nc.vector.tensor_reduce(
                out=nmx_all[:, m : m + 1],
                in_=flat,
                axis=mybir.AxisListType.X,
                op=mybir.AluOpType.max,
                negate=True,
            )
            # exp(scores - max) with fused per-row sum on the ACT engine
            expt = scratch.tile([P, CLS], f32, name="expt")
            nc.scalar.activation(
                out=expt,
                in_=flat,
                func=mybir.ActivationFunctionType.Exp,
                bias=nmx_all[:, m : m + 1],
                scale=1.0,
                accum_out=sume_all[:, m : m + 1],
            )
            # s_target = sum_c scores[c] * (iota[c] == target), one DVE pass
            junk = scratch.tile([P, CLS], f32, name="junk")
            nc.vector.scalar_tensor_tensor(
                out=junk,
                in0=iota_f,
                scalar=tgt_sb[:, m : m + 1],
                in1=flat,
                op0=mybir.AluOpType.is_equal,
                op1=mybir.AluOpType.mult,
                accum_out=st_all[:, m : m + 1],
            )

        m0 += sub

    # tail: nll = (log(sum) - (-max)) - s_target = lse - s_target
    lsum = singles.tile([P, MT], f32, name="lsum")
    nc.scalar.activation(
        out=lsum, in_=sume_all, func=mybir.ActivationFunctionType.Ln
    )
    for m in range(MT):
        nc.vector.scalar_tensor_tensor(
            out=nll_sb[:, m : m + 1],
            in0=lsum[:, m : m + 1],
            scalar=nmx_all[:, m : m + 1],
            in1=st_all[:, m : m + 1],
            op0=mybir.AluOpType.subtract,
            op1=mybir.AluOpType.subtract,
        )
    nc.sync.dma_start(out=nll.ap(), in_=nll_sb)


def prepare_inputs(feature, anchor, _target, mm_dtype: str = MM_DTYPE):
    """Host-side sharding + layout prep. Returns per-core input maps.

    featX[g, kt, p, c] = feature_core[g*GCOLS + c, kt*P + p] so every
    per-(g,kt) device DMA reads one fully contiguous HBM block.
    """
    npdt = _np_mm(mm_dtype)
    feature = np.asarray(feature, dtype=np.float32)
    anchor = np.asarray(anchor, dtype=np.float32)
    tgt_f = np.asarray(_target).astype(np.float32)

    dr2 = DR_MODE == 2 and mm_dtype == "f8"
    anchorT = np.ascontiguousarray(anchor.T).astype(npdt)  # [FEAT, CLS]
    in_maps = []
    for c in range(NCORES):
        sl = slice(c * BPC, (c + 1) * BPC)
        if dr2:
            # SwInterleave weight layout: per (g, kt-pair, partition), each
            # 128-col m-tile stores [A127 B127 A126 B126 ... A0 B0] where
            # A/B are the kt-pair planes and columns run in reverse.
            ff = feature[sl].reshape(NGRP, MPG, P, KT // 2, 2, P)
            fx = ff.transpose(0, 3, 5, 1, 2, 4)[:, :, :, :, ::-1, :]
            featX_c = np.ascontiguousarray(
                fx.reshape(NGRP, KT // 2, P, 2 * GCOLS)
            ).astype(npdt)
        else:
            # [BPC, FEAT] -> [NGRP, GCOLS, KT, P] -> [NGRP, KT, P, GCOLS]
            fx = feature[sl].reshape(NGRP, GCOLS, KT, P).transpose(0, 2, 3, 1)
            featX_c = np.ascontiguousarray(fx).astype(npdt)
        tgtX_c = np.ascontiguousarray(tgt_f[sl].reshape(MT, P).T)  # [P, MT]
        in_maps.append(
            {
                "featX": featX_c,
                "anchorT": anchorT,
                "tgtX": tgtX_c,
            }
        )
    return in_maps


def unpack_nll(arr: np.ndarray) -> np.ndarray:
    """Device nll layout [P, MT] (arr[p, m] = row m*P + p) -> flat [BPC]."""
    return np.ascontiguousarray(arr.T).reshape(BPC)


_PROGRAM_CACHE: dict = {}


def _get_program(mm_dtype: str, reps: int = 1, variant: str = "hostT") -> bass.Bass:
    key = (mm_dtype, reps, variant)
    nc = _PROGRAM_CACHE.get(key)
    if nc is None:
        if variant.startswith("loop"):
            nc = build_program(mm_dtype, loop_iters=int(variant[4:]))
        else:
            nc = build_program(mm_dtype, reps=reps)
        nc.compile()  # bacc pass pipeline (reg alloc, wait splitting, ...)
        _PROGRAM_CACHE[key] = nc
    return nc


_RUNNER_CACHE: dict = {}


def make_runner(nc: bass.Bass, in_maps):
    """Compile once; return callable that re-executes with device-resident
    inputs (only the tiny donated output zeros are re-created per call)."""
    import jax
    import jax.core
    from jax.experimental.shard_map import shard_map
    from jax.sharding import Mesh, NamedSharding, PartitionSpec

    from concourse import bass2jax, mybir as mb

    bass2jax.install_neuronx_cc_hook()

    partition_name = (
        nc.partition_id_tensor.name if nc.partition_id_tensor else None
    )
    in_names, out_names, out_avals, zero_shapes = [], [], [], []
    for alloc in nc.m.functions[0].allocations:
        if not isinstance(alloc, mb.MemoryLocationSet):
            continue
        name = alloc.memorylocations[0].name
        if alloc.kind == "ExternalInput":
            if name != partition_name:
                in_names.append(name)
        elif alloc.kind == "ExternalOutput":
            shape = tuple(alloc.tensor_shape)
            dtype = mb.dt.np(alloc.dtype)
            out_names.append(name)
            out_avals.append(jax.core.ShapedArray(shape, dtype))
            zero_shapes.append((shape, dtype))
    n_params = len(in_names)
    n_outs = len(out_names)
    all_in_names = list(in_names) + list(out_names)
    if partition_name is not None:
        all_in_names.append(partition_name)

    donate = tuple(range(n_params, n_params + n_outs))

    def _body(*args):
        operands = list(args)
        if partition_name is not None:
            operands.append(bass2jax.partition_id_tensor())
        outs = bass2jax._bass_exec_p.bind(
            *operands,
            out_avals=tuple(out_avals),
            in_names=tuple(all_in_names),
            out_names=tuple(out_names),
            lowering_input_output_aliases=(),
            sim_require_finite=True,
            sim_require_nnan=True,
            nc=nc,
        )
        return tuple(outs)

    devices = jax.devices()[:NCORES]
    mesh = Mesh(np.asarray(devices), ("core",))
    in_specs = (PartitionSpec("core"),) * (n_params + n_outs)
    out_specs = (PartitionSpec("core"),) * n_outs
    sharded = jax.jit(
        shard_map(
            _body, mesh=mesh, in_specs=in_specs, out_specs=out_specs,
            check_rep=False,
        ),
        donate_argnums=donate,
        keep_unused=True,
    )
    sharding = NamedSharding(mesh, PartitionSpec("core"))
    dev_in = [
        jax.device_put(
            np.concatenate([np.asarray(in_maps[c][nm]) for c in range(NCORES)], axis=0),
            sharding,
        )
        for nm in in_names
    ]
    jax.block_until_ready(dev_in)

    def run():
        zeros = [
            np.zeros((NCORES * s[0], *s[1:]), dt) for (s, dt) in zero_shapes
        ]
        outs = sharded(*dev_in, *zeros)
        jax.block_until_ready(outs)
        return {
            nm: np.asarray(outs[i]).reshape(NCORES, *out_avals[i].shape)
            for i, nm in enumerate(out_names)
        }

    return run


def timed_run(in_maps, mm_dtype: str = MM_DTYPE, reps: int = 1, iters: int = 3,
              variant: str = "hostT"):
    """Compile the reps-times-repeated program, return best wall seconds/call."""
    import time

    key = (mm_dtype, reps, variant, id(in_maps))
    runner = _RUNNER_CACHE.get(key)
    if runner is None:
        nc = _get_program(mm_dtype, reps=reps, variant=variant)
        runner = make_runner(nc, in_maps)
        _RUNNER_CACHE[key] = runner
    runner()  # warmup (compile + first exec)
    best = float("inf")
    for _ in range(iters):
        t0 = time.perf_counter()
        runner()
        best = min(best, time.perf_counter() - t0)
    return best


def run_on_cores(in_maps, mm_dtype: str = MM_DTYPE, trace: bool = False):
    from concourse.bass_utils import run_bass_kernel_spmd

    nc = _get_program(mm_dtype)
    res = run_bass_kernel_spmd(nc, in_maps, list(range(NCORES)), trace=trace)
    return res


def kernel(feature, anchor, _target) -> np.ndarray:
    mm_dtype = MM_DTYPE
    in_maps = prepare_inputs(feature, anchor, _target, mm_dtype)
    res = run_on_cores(in_maps, mm_dtype, trace=os.environ.get("CL_TRACE", "") == "1")
    nll_all = np.concatenate(
        [unpack_nll(res.results[c]["nll"]) for c in range(NCORES)]
    )
    if os.environ.get("CL_TRACE", "") == "1" and res.exec_time_ns is not None:
        print(f"HW exec time: {res.exec_time_ns} ns")
    return np.asarray(np.mean(nll_all, dtype=np.float64), dtype=np.float32)
